# revision 1
# baseline (speedup 1.0000x reference)
"""CTC loss (mean reduction) on 8 Trainium2 NeuronCores.

Strategy: data-parallel over batch (4 utterances per core). The CTC lattice
DP runs in the linear probability domain with per-utterance exponential
rescaling (emissions multiplied by e^rr so alpha stays in fp32 range).
Serial-in-T dependency is folded into hardware `tensor_tensor_scan`
instructions: for each extended-label state s (wavefront over 257 levels),
alpha[s, :] over all T is one first-order recurrence
    x_t = p_t * x_{t-1} + c_t,   c_t = p_t * (A[s-1]_{t-1} + m3[s]*A[s-2]_{t-1})
computed as 32 parallel t-chunks on partitions (4 b x 32 chunks = 128
partitions, 32 time steps per chunk on the free axis), with the exact
chunk-boundary recurrence solved by a second (transposed) scan over chunks.
"""

import numpy as np

import concourse.bass as bass
import concourse.mybir as mybir
from concourse.bass_utils import run_bass_kernel_spmd

B, T, C, U = 32, 1000, 1024, 128
S = 2 * U + 1          # 257 extended states
NCORES = 8
BPC = B // NCORES      # 4 utterances per core
K = 32                 # time chunks (partition dim = BPC * K = 128)
CW = 32                # chunk width (time steps per chunk)
TP = K * CW            # padded T = 1024
W = CW + 1             # stored row width: 1 halo col + CW
ROWS = S + 2           # 2 virtual zero rows below level 0
ROW0 = 129            # first row shipped back (min needed row = 2*64+1-... = 129)
OUT_W = (ROWS - ROW0) * W
DELTA = -2.44          # rate-proxy calibration offset (see _rates)
F32 = mybir.dt.float32
OP = mybir.AluOpType


def _build_nc():
    nc = bass.Bass()
    ptil = nc.declare_dram_parameter("ptil", [128, S * CW], F32, isOutput=False)
    m3sc = nc.declare_dram_parameter("m3sc", [128, S], F32, isOutput=False)
    rhod = nc.declare_dram_parameter("rhod", [128, 1], F32, isOutput=False)
    outd = nc.declare_dram_parameter("outd", [128, OUT_W], F32, isOutput=True)

    with (
        nc.Block() as block,
        nc.semaphore("in_sem") as in_sem,
        nc.semaphore("done") as done,
        nc.sbuf_tensor("P", [128, S * CW], F32) as P,
        nc.sbuf_tensor("M3", [128, S], F32) as M3,
        nc.sbuf_tensor("A", [128, ROWS * W], F32) as A,
        nc.sbuf_tensor("d_t", [128, CW], F32) as d_t,
        nc.sbuf_tensor("c_t", [128, CW], F32) as c_t,
        nc.sbuf_tensor("u_t", [128, CW], F32) as u_t,
        nc.sbuf_tensor("g_t", [128, CW], F32) as g_t,
        nc.sbuf_tensor("zz", [128, CW], F32) as zz,
        nc.sbuf_tensor("stageU", [128, 32], F32) as stageU,
        nc.sbuf_tensor("stageG", [128, 32], F32) as stageG,
        nc.sbuf_tensor("stageUT", [128, 32], F32) as stageUT,
        nc.sbuf_tensor("stageGT", [128, 32], F32) as stageGT,
        nc.sbuf_tensor("xrow", [128, 33], F32) as xrow,
        nc.sbuf_tensor("xcol", [128, 32], F32) as xcol,
        nc.sbuf_tensor("initv", [128, 1], F32) as initv,
        nc.sbuf_tensor("RH", [128, 1], F32) as RH,
    ):
        @block.sync
        def _(sync):
            sync.dma_start(out=P[:, :], in_=ptil[:, :]).then_inc(in_sem, 16)
            sync.dma_start(out=M3[:, :], in_=m3sc[:, :]).then_inc(in_sem, 16)
            sync.dma_start(out=RH[:, :], in_=rhod[:, :]).then_inc(in_sem, 16)
            sync.wait_ge(done, 1)
            sync.dma_start(
                out=outd[:, :], in_=A[:, ROW0 * W : ROWS * W]
            ).then_inc(in_sem, 16)

        @block.vector
        def _(vector):
            v = vector
            v.memset(A[:, 0 : 2 * W], 0.0)       # virtual levels -2, -1
            v.memset(zz[:, :], 0.0)
            v.memset(xrow[:, :], 0.0)
            v.memset(stageU[:, :], 0.0)
            v.memset(stageG[:, :], 0.0)
            v.wait_ge(in_sem, 48)
            last = None
            for s in range(S):
                r = s + 2
                if s == 0:
                    v.memset(initv[:, :], 1.0)   # alpha_{t=-1}[0] = 1
                    v.memset(xrow[:, 0:1], 1.0)
                elif s == 1:
                    v.memset(initv[:, :], 0.0)
                    v.memset(xrow[:, 0:1], 0.0)
                a1 = A[:, (r - 1) * W : (r - 1) * W + CW]  # level s-1, t-1 view
                a2 = A[:, (r - 2) * W : (r - 2) * W + CW]  # level s-2, t-1 view
                p_s = P[:, s * CW : (s + 1) * CW]
                # d = a1 + m3[s] * a2   (per-partition scalar m3)
                v.scalar_tensor_tensor(
                    d_t[:, :], a2, M3[:, s : s + 1], a1, OP.mult, OP.add
                )
                v.scalar_tensor_tensor(
                    c_t[:, :], d_t[:, :], RH[:, 0:1], p_s, OP.mult, OP.mult
                )
                # chunk-local particular solution and homogeneous coefficient
                v.tensor_tensor_scan(
                    u_t[:, :], p_s, c_t[:, :], 0.0, OP.mult, OP.add
                )
                v.tensor_tensor_scan(
                    g_t[:, :], p_s, zz[:, :], 1.0, OP.mult, OP.add
                )
                # chunk tails -> col 0 of staging; transpose -> rows 32b
                v.tensor_copy(stageU[:, 0:1], u_t[:, CW - 1 : CW])
                v.tensor_copy(stageG[:, 0:1], g_t[:, CW - 1 : CW])
                v.transpose(stageUT[:, 0:32], stageU[:, 0:32])
                v.transpose(stageGT[:, 0:32], stageG[:, 0:32])
                # boundary recurrence across chunks: S_k = g_e[k]*S_{k-1} + u_e[k]
                for bq in range(BPC):
                    q = 32 * bq
                    v.tensor_tensor_scan(
                        xrow[q : q + 1, 1:33],
                        stageGT[q : q + 1, 0:32],
                        stageUT[q : q + 1, 0:32],
                        initv[q : q + 1, 0:1],
                        OP.mult,
                        OP.add,
                    )
                v.transpose(xcol[:, 0:32], xrow[:, 0:32])
                # exact row: x = u + g * X  (X = alpha at own chunk start - 1)
                row = A[:, r * W : (r + 1) * W]
                last = v.scalar_tensor_tensor(
                    row[:, 1 : 1 + CW], g_t[:, :], xcol[:, 0:1], u_t[:, :],
                    OP.mult, OP.add,
                )
                last = v.tensor_copy(row[:, 0:1], xcol[:, 0:1])
            last.then_inc(done, 1)

    return nc


_NC_CACHE = None
_LAST_IN_MAPS = None


def kernel(log_probs, targets, input_lengths, target_lengths):
    global _NC_CACHE
    lp = np.asarray(log_probs, np.float32)
    tg = np.asarray(targets, np.int32)
    il = np.asarray(input_lengths, np.int32)
    tl = np.asarray(target_lengths, np.int32)

    # extended sequence (blank,l1,blank,...), skip mask, gathered log-emissions
    ext = np.zeros((B, S), np.int32)
    ext[:, 1::2] = tg
    prev2 = np.concatenate([np.zeros((B, 2), np.int32), ext[:, :-2]], axis=1)
    m3 = ((ext != 0) & (ext != prev2)).astype(np.float32)
    E = np.take_along_axis(lp, ext[:, None, :], axis=2)  # [B,T,S]

    # scaling metadata: per-t rescale increments + per-level tilt, from a
    # float64 normalized host pass (also yields an exact t* row for fallback)
    sl = 2 * tl
    E64 = E.astype(np.float64)
    NEG = -1e30
    RGRID = np.array([0.0, 0.1, 0.25, 0.4, 0.55, 0.7, 0.85, 1.0])
    ss = np.arange(S)
    cone = ss[None, :] <= sl[:, None]
    tiltmat = RGRID[None, :, None] * ss[None, None, :]
    a = np.full((B, S), NEG); a[:, 0] = E64[:, 0, 0]; a[:, 1] = E64[:, 0, 1]
    mt = np.full((B, T, len(RGRID)), NEG)
    snap = np.zeros((B, S))
    m3b = m3 > 0
    for t in range(T):
        if t > 0:
            a2 = np.concatenate([np.full((B, 1), NEG), a[:, :-1]], axis=1)
            a3 = np.where(m3b, np.concatenate([np.full((B, 2), NEG), a[:, :-2]], axis=1), NEG)
            m = np.maximum(np.maximum(a, a2), a3)
            a = m + np.log(np.exp(a - m) + np.exp(a2 - m) + np.exp(a3 - m)) + E64[:, t, :]
        tilted = np.where(cone[:, None, :], a[:, None, :] - tiltmat, NEG)
        mt[:, t] = tilted.max(axis=2)
        hit = (il - 1) == t
        if hit.any():
            snap[hit] = a[hit]
    rho_i = np.zeros(B, np.int64)
    for b in range(B):
        vt = np.where(cone[b], snap[b], NEG)
        smax = int(np.argmax(vt)); va = max(vt[sl[b]], vt[sl[b] - 1])
        want = max(0.0, (vt[smax] - va) / max(sl[b] - smax, 1)) if smax < sl[b] - 1 else 0.0
        rho_i[b] = int(np.argmin(np.abs(RGRID - want)))
    rho = RGRID[rho_i]
    r = np.zeros((B, T))
    for b in range(B):
        ts = int(il[b])
        mx = mt[b, :ts, rho_i[b]]
        r[b, 0] = -mx[0]; r[b, 1:ts] = mx[:-1] - mx[1:]

    logptil = np.full((B, TP, S), -200.0, np.float32)
    for b in range(B):
        tb = int(il[b])
        logptil[b, :tb, :] = E[b, :tb, :] + r[b, :tb, None]
    ptil = np.exp(logptil, dtype=np.float32)          # [B,TP,S]
    # device layout [b*K + k partition, s*CW + j free]
    ptil = ptil.reshape(B, K, CW, S).transpose(0, 1, 3, 2)  # [B,K,S,CW]
    ptil = np.ascontiguousarray(ptil.reshape(B, K, S * CW))

    m3t = m3 * np.exp(-rho)[:, None].astype(np.float32)
    m3sc = np.repeat(m3t[:, None, :], K, axis=1)      # [B,K,S]
    rhosc = np.exp(-rho).astype(np.float32)
    rhop = np.repeat(rhosc[:, None], K, axis=1).reshape(B, K, 1)

    if _NC_CACHE is None:
        _NC_CACHE = _build_nc()
    nc = _NC_CACHE

    in_maps = []
    for c in range(NCORES):
        bs = slice(c * BPC, (c + 1) * BPC)
        in_maps.append({
            "ptil": ptil[bs].reshape(128, S * CW),
            "m3sc": np.ascontiguousarray(m3sc[bs].reshape(128, S)),
            "rhod": np.ascontiguousarray(rhop[bs].reshape(128, 1)),
        })
    global _LAST_IN_MAPS
    _LAST_IN_MAPS = in_maps
    res = run_bass_kernel_spmd(nc, in_maps, core_ids=list(range(NCORES)))

    lls = np.zeros(B, np.float64)
    for b in range(B):
        core, bloc = b // BPC, b % BPC
        outd = res.results[core]["outd"]              # [128, OUT_W]
        tstar = int(il[b]) - 1
        kstar, jstar = tstar // CW, tstar % CW
        p = bloc * K + kstar
        slb = int(sl[b])
        v1 = float(outd[p, (slb + 2 - ROW0) * W + 1 + jstar])
        v2 = float(outd[p, (slb + 1 - ROW0) * W + 1 + jstar])
        corr = -r[b, : il[b]].sum()
        if np.isfinite(v1 + v2) and (v1 > 0 or v2 > 0):
            l1 = np.log(max(v1, 1e-300)) + rho[b] * slb + corr
            l2 = np.log(max(v2, 1e-300)) + rho[b] * (slb - 1) + corr
            lls[b] = np.logaddexp(l1, l2)
        else:
            lls[b] = np.logaddexp(snap[b, slb], snap[b, slb - 1])
    loss = -lls.sum() / il.astype(np.float64).sum()
    return np.float32(loss)



# revision 4
# speedup vs baseline: 3.6306x; 3.6306x over previous
"""CTC loss (mean reduction) on 8 Trainium2 NeuronCores.

Data-parallel over batch: 4 utterances per core, one partition each, with the
S=257 extended-label states on the free axis. The lattice DP runs t-major in
the linear-probability domain in fp32:

    A_t[s] = (A_{t-1}[s] + e^{-g} A_{t-1}[s-1] + m3[s] e^{-2g} A_{t-1}[s-2]) * p_t[s]

Range control needs no host-side DP oracle:
  * emissions are shipped as fp8 (e4m3) of exp(E + SHIFT_b), SHIFT_b a
    per-utterance constant;
  * an exact per-utterance "tilt" e^{-g*s} (g from a fitted function of the
    advance rate sl/il) is folded into the transition weights so the renorm
    max tracks the answer diagonal;
  * the device renormalizes by 1/max every RN=8 steps (reciprocal factors are
    shipped back and log-summed on the host).
Each utterance's emission stream is padded past t=il-1 with a "freeze"
pattern (p[sl]=1, else 0) whose first step computes alpha[sl]+alpha[sl-1]
exactly -- the CTC log-likelihood lands in the final alpha, so no mid-stream
snapshot is needed.

Engines: activation does all DMAs + fp8->f32 upcasts (fully unrolled, static
offsets); vector runs the DP with a hardware Fori loop over chunk pairs
(32 time steps per chunk), double-buffered via semaphores.
"""

import numpy as np
import ml_dtypes

import concourse.bass as bass
import concourse.mybir as mybir
from concourse.bass_utils import run_bass_kernel_spmd

B, T, C, U = 32, 1000, 1024, 128
S = 2 * U + 1            # 257 extended states
NCORES = 8
BPC = B // NCORES        # 4 utterances per core
CT = 32                  # time steps per chunk
NCH = 33                 # chunks (odd; chunk 0 unrolled, rest looped in pairs)
TP = NCH * CT            # padded time 1056
NIT = (NCH - 1) // 2     # Fori iterations
RN = 8                   # renorm period (steps)
CW = CT * S              # chunk width in elements 8224
OB = 260                 # outd column where renorm factors start
NRF = 4 * NCH            # total renorm factors 132
OUTW = OB + 4 + 8 * NIT  # 392
F32 = mybir.dt.float32
F8 = mybir.dt.float8e4
F8NP = mybir.dt.np(F8)
OP = mybir.AluOpType
AX = mybir.AxisListType
# tilt fit: g = polyval(GCO, sl/il), calibrated on the input distribution
GCO = (4.0775, -6.8982, 3.1779)


def _build_nc(detect_races=True):
    nc = bass.Bass(detect_race_conditions=detect_races)
    pt = nc.declare_dram_parameter("pt", [BPC, TP * S], F8, isOutput=False)
    m3eg = nc.declare_dram_parameter("m3eg", [BPC, S + 1], F32, isOutput=False)
    outd = nc.declare_dram_parameter("outd", [BPC, OUTW], F32, isOutput=True)

    with (
        nc.semaphore("s_in") as s_in,
        nc.semaphore("s_rdy") as s_rdy,
        nc.semaphore("s_free") as s_free,
        nc.semaphore("s_out") as s_out,
        nc.sbuf_tensor("t8A", [BPC, CW], F8) as t8A,
        nc.sbuf_tensor("t8B", [BPC, CW], F8) as t8B,
        nc.sbuf_tensor("fA", [BPC, CW], F32) as fA,
        nc.sbuf_tensor("fB", [BPC, CW], F32) as fB,
        nc.sbuf_tensor("M3", [BPC, S + 1], F32) as M3,
        nc.sbuf_tensor("AE", [BPC, S + 2], F32) as AE,
        nc.sbuf_tensor("AO", [BPC, S + 2], F32) as AO,
        nc.sbuf_tensor("s1t", [BPC, S], F32) as s1t,
        nc.sbuf_tensor("a3t", [BPC, S], F32) as a3t,
        nc.sbuf_tensor("s2t", [BPC, S], F32) as s2t,
        nc.sbuf_tensor("mtmp", [BPC, 1], F32) as mtmp,
        nc.sbuf_tensor("stg", [BPC, 8], F32) as stg,
        nc.sbuf_tensor("stg0", [BPC, 4], F32) as stg0,
    ):
        act = nc.scalar
        vec = nc.vector

        def chunk_ap(c):
            return pt[:, c * CW : (c + 1) * CW]

        # ---------------- activation engine: DMAs + upcasts (unrolled) ------
        act.dma_start(out=M3[:, :], in_=m3eg[:, :]).then_inc(s_in, 16)   # 1
        act.dma_start(out=t8B[:, :], in_=chunk_ap(0)).then_inc(s_in, 16)  # 2
        act.dma_start(out=t8A[:, :], in_=chunk_ap(1)).then_inc(s_in, 16)  # 3
        act.wait_ge(s_in, 48)
        act.copy(fB[:, :], t8B[:, :]).then_inc(s_rdy, 1)   # c0 ready
        act.dma_start(out=t8B[:, :], in_=chunk_ap(2)).then_inc(s_in, 16)  # 4
        act.copy(fA[:, :], t8A[:, :]).then_inc(s_rdy, 1)   # c1 ready
        act.dma_start(out=t8A[:, :], in_=chunk_ap(3)).then_inc(s_in, 16)  # 5
        for k in range(2, NCH):
            t8X, fX = (t8A, fA) if k % 2 else (t8B, fB)
            act.wait_ge(s_in, 16 * (k + 2))
            act.wait_ge(s_free, k - 1)
            act.copy(fX[:, :], t8X[:, :]).then_inc(s_rdy, 1)  # c_k ready
            if k + 2 < NCH:
                act.dma_start(out=t8X[:, :], in_=chunk_ap(k + 2)).then_inc(
                    s_in, 16
                )
            if k == 2:
                act.dma_start(out=outd[:, OB : OB + 4], in_=stg0[:, :]).then_inc(
                    s_out, 16
                )
            if k >= 4 and k % 2 == 0:
                i = (k - 4) // 2
                act.dma_start(
                    out=outd[:, OB + 4 + 8 * i : OB + 12 + 8 * i], in_=stg[:, :]
                ).then_inc(s_out, 16)
        act.wait_ge(s_free, NCH)
        act.dma_start(
            out=outd[:, OB + 4 + 8 * (NIT - 1) : OUTW], in_=stg[:, :]
        ).then_inc(s_out, 16)
        act.dma_start(out=outd[:, 0 : S + 2], in_=AO[:, :]).then_inc(s_out, 16)
        act.wait_ge(s_out, 16 * (NIT + 2))

        # ---------------- vector engine: the DP ----------------------------
        def step(src, dst, pf, lt, stgt, cbase):
            # s1 = e^{-g} * A[s-1] + A[s]
            vec.scalar_tensor_tensor(
                s1t[:, :], src[:, 1 : 1 + S], M3[:, S : S + 1],
                src[:, 2 : 2 + S], OP.mult, OP.add,
            )
            # a3 = (m3 * e^{-2g}) * A[s-2]
            vec.tensor_tensor(a3t[:, :], src[:, 0:S], M3[:, 0:S], OP.mult)
            vec.tensor_tensor(s2t[:, :], s1t[:, :], a3t[:, :], OP.add)
            last = vec.tensor_tensor(
                dst[:, 2 : 2 + S], s2t[:, :], pf[:, lt * S : (lt + 1) * S],
                OP.mult,
            )
            if lt % RN == RN - 1:
                col = cbase + lt // RN
                vec.tensor_reduce(mtmp[:, :], dst[:, 2 : 2 + S], AX.X, OP.max)
                vec.reciprocal(stgt[:, col : col + 1], mtmp[:, :])
                last = vec.tensor_scalar_mul(
                    dst[:, 2 : 2 + S], dst[:, 2 : 2 + S], stgt[:, col : col + 1]
                )
            return last

        # guards stay zero forever; AE body is re-zeroed where the t=0 init
        # does not write; AO body is fully written by the first step.
        vec.memset(AE[:, 0:2], 0.0)
        vec.memset(AO[:, 0:2], 0.0)
        vec.memset(AE[:, 4 : S + 2], 0.0)
        vec.wait_ge(s_rdy, 1)
        vec.tensor_copy(AE[:, 2:4], fB[:, 0:2])   # t=0 init (tilt pre-baked)
        last = None
        for lt in range(1, CT):                   # chunk 0: steps 1..31
            src, dst = (AO, AE) if lt % 2 == 0 else (AE, AO)
            last = step(src, dst, fB, lt, stg0, 0)
        last.then_inc(s_free, 1)

        rR = vec.alloc_register("rR")
        rO = vec.alloc_register("rO")
        vec.reg_mov(rR, 1)
        vec.reg_mov(rO, 0)
        with vec.Fori(0, NIT):
            vec.reg_add(rO, rO, 16)
            vec.wait_ge(s_out, rO)
            for half, fX in ((0, fA), (1, fB)):
                vec.reg_add(rR, rR, 1)
                vec.wait_ge(s_rdy, rR)
                last = None
                for lt in range(CT):
                    src, dst = (AO, AE) if lt % 2 == 0 else (AE, AO)
                    last = step(src, dst, fX, lt, stg, 4 * half)
                last.then_inc(s_free, 1)

    return nc


_NC_CACHE = None
_LAST_IN_MAPS = None


def _prep(lp, tg, il, tl):
    """Host-side emission prep. Returns (in_maps, g, shift, sl)."""
    ext = np.zeros((B, S), np.int32)
    ext[:, 1::2] = tg
    prev2 = np.concatenate([np.zeros((B, 2), np.int32), ext[:, :-2]], axis=1)
    m3 = ((ext != 0) & (ext != prev2)).astype(np.float32)
    E = np.take_along_axis(lp, ext[:, None, :], axis=2)      # [B,T,S] f32
    sl = (2 * tl).astype(np.int64)

    nu = sl / il
    g = np.polyval(GCO, nu)
    g = np.clip(g, 0.2, 3.5).astype(np.float64)

    # per-utterance shift so exp(E + shift) fits fp8 e4m3 (max ~240)
    Emax = E.max(axis=(1, 2)).astype(np.float64)
    shift = np.minimum(7.5, 5.0 - Emax)

    p8 = np.zeros((B, TP, S), F8NP)
    for b in range(B):
        ib = int(il[b])
        pf = np.exp(E[b, :ib].astype(np.float64) + shift[b])
        pf[0, 1] *= np.exp(-g[b])          # tilt on the t=0 init of state 1
        p8[b, :ib] = np.minimum(pf, 224.0).astype(F8NP)
        p8[b, ib:, sl[b]] = 1.0            # freeze pattern
    m3eg = np.zeros((B, S + 1), np.float32)
    m3eg[:, :S] = m3 * np.exp(-2 * g)[:, None]
    m3eg[:, S] = np.exp(-g)

    in_maps = []
    for c in range(NCORES):
        bs = slice(c * BPC, (c + 1) * BPC)
        in_maps.append({
            "pt": np.ascontiguousarray(p8[bs].reshape(BPC, TP * S)),
            "m3eg": np.ascontiguousarray(m3eg[bs]),
        })
    return in_maps, g, shift, sl, ext, m3


def _ll_exact(lp, ext, m3, il, sl, bsel):
    """Float64 log-domain DP fallback for utterances in bsel."""
    nb = len(bsel)
    E = np.take_along_axis(
        lp[bsel].astype(np.float64), ext[bsel][:, None, :], axis=2)
    NEGL = -1e30
    a = np.full((nb, S), NEGL)
    a[:, 0] = E[:, 0, 0]
    a[:, 1] = E[:, 0, 1]
    m3b = m3[bsel] > 0
    snap = np.zeros((nb, S))
    ilb = il[bsel]
    for t in range(int(ilb.max())):
        if t > 0:
            a2 = np.concatenate([np.full((nb, 1), NEGL), a[:, :-1]], axis=1)
            a3 = np.where(
                m3b,
                np.concatenate([np.full((nb, 2), NEGL), a[:, :-2]], axis=1),
                NEGL,
            )
            m = np.maximum(np.maximum(a, a2), a3)
            a = m + np.log(
                np.exp(a - m) + np.exp(a2 - m) + np.exp(a3 - m)
            ) + E[:, t, :]
        hit = (ilb - 1) == t
        if hit.any():
            snap[hit] = a[hit]
    slb = sl[bsel]
    r = np.arange(nb)
    return np.logaddexp(snap[r, slb], snap[r, slb - 1])


def kernel(log_probs, targets, input_lengths, target_lengths):
    global _NC_CACHE, _LAST_IN_MAPS
    lp = np.asarray(log_probs, np.float32)
    tg = np.asarray(targets, np.int32)
    il = np.asarray(input_lengths, np.int64)
    tl = np.asarray(target_lengths, np.int64)

    in_maps, g, shift, sl, ext, m3 = _prep(lp, tg, il, tl)
    if _NC_CACHE is None:
        _NC_CACHE = _build_nc()
    _LAST_IN_MAPS = in_maps
    res = run_bass_kernel_spmd(_NC_CACHE, in_maps, core_ids=list(range(NCORES)))

    ll = np.zeros(B, np.float64)
    bad = []
    for b in range(B):
        core, row = b // BPC, b % BPC
        o = res.results[core]["outd"][row].astype(np.float64)
        afin = o[2 + sl[b]]
        rhat = o[OB:OUTW]
        if afin > 0 and np.all(rhat > 0) and np.all(np.isfinite(rhat)):
            ll[b] = (np.log(afin) - np.log(rhat).sum()
                     - shift[b] * il[b] + g[b] * sl[b])
        else:
            bad.append(b)
    if bad:
        ll[bad] = _ll_exact(lp, ext, m3, il, sl, np.array(bad))
    loss = -ll.sum() / il.sum()
    return np.float32(loss)
